# revision 6
# baseline (speedup 1.0000x reference)
"""Trainium2 Bass kernel for InvertedResidual (ShuffleNetV2 stride-1 unit).

Reference computation (per image, NCHW, C=232):
    x1, x2 = split(x, 116)
    h  = prelu(bn1(conv1x1(x2, w2)), a1)
    h2 = bn2(dwconv3x3(h, wdw))
    y  = prelu(bn3(conv1x1(h2, w3)), a2)
    out = channel_shuffle(concat(x1, y))   # out[2j]=x1[j], out[2j+1]=y[j]

v2 design (data-parallel over batch=64 on 8 cores, 8 images/core):
  - Free dim is IMAGE-INTERLEAVED: host ships x2 as [116, 3136, 8] bf16
    (channel on partitions, pixel-major / image-minor on free dim). Each
    DMA descriptor then covers all 8 images of one channel slice -> 8x
    fewer, 8x bigger descriptors than per-image layout. bf16 halves bytes.
  - x1 passthrough + channel shuffle are pure data movement -> host-side
    numpy during unshard (device computes only the y = branch2 half).
  - conv1x1 #1 (BN1-folded, bf16 weights) as 56 row-matmuls [116x448]
    (448 = 56 px * 8 img <= 512 PSUM bank); PReLU via ScalarE -> padded
    h1 [58 rows x 58 cols x 8 img] with zeroed borders (gpsimd memsets).
  - dw3x3+BN2+conv1x1#2+BN3 fused: 9 taps as dense 116x116 bf16 matmuls
    over shifted h1 row-slices, accumulated in PSUM per output row.
    PSUM budget: 2 banks conv1 double-buffer + 6 banks tap accumulators.
  - Emission is software-pipelined: conv1 blocks run 3 groups ahead of
    the tap groups that consume them, so PE never stalls on ACT.
  - Output y stored bf16 [116, 25088]; host converts/interleaves to f32.
"""

import numpy as np
import ml_dtypes

EPS = 1e-5
NCORES = 8
NIMG = 8            # images per core
BF = 116            # branch features
H = W = 56
HW = H * W          # 3136
FREE = HW * NIMG    # 25088 free-dim elems per partition
ROWF = W * NIMG     # 448 free-dim elems per output row (matmul width)
PW = 58             # padded row width
PROWF = PW * NIMG   # 464 padded free-dim elems per h1 row
RPG = 6             # output rows per PSUM tap group (6 accumulator banks)
NLD = 4             # x2 load chunks
NST = 4             # y store chunks

TAP_ORDER = [(dy, dx) for dy in range(3) for dx in range(3)]

_CACHE = {}


def _build(alpha1: float, alpha2: float, reps: int = 1, variant: str = "full"):
    import concourse.mybir as mybir
    import concourse.tile as tile
    from concourse import bacc

    f32 = mybir.dt.float32
    bf16 = mybir.dt.bfloat16
    PRELU = mybir.ActivationFunctionType.Prelu

    nc = bacc.Bacc("TRN2", target_bir_lowering=False, debug=False)
    x2 = nc.dram_tensor("x2", [BF, FREE], bf16, kind="ExternalInput")
    wts = nc.dram_tensor("wts", [BF, 10 * BF], bf16, kind="ExternalInput")
    bias = nc.dram_tensor("bias", [BF, 2], f32, kind="ExternalInput")
    out = nc.dram_tensor("out", [BF, FREE], bf16, kind="ExternalOutput")

    # store DMAs from ACT's HWDGE queue set, loads from SP's
    st_engine = "scalar" if variant == "actst" else "sync"

    groups = []
    r0 = 0
    while r0 < H:
        n = min(RPG, H - r0)
        groups.append((r0, n))
        r0 += n

    with tile.TileContext(nc) as tc:
        with (
            tc.tile_pool(name="const", bufs=1) as constp,
            tc.tile_pool(name="ps1", bufs=2, space="PSUM") as ps1p,
            tc.tile_pool(name="ps2", bufs=RPG, space="PSUM") as ps2p,
        ):
            wsb = constp.tile([BF, 10 * BF], bf16)
            nc.sync.dma_start(wsb[:], wts[:, :])
            bsb = constp.tile([BF, 2], f32)
            nc.sync.dma_start(bsb[:], bias[:, :])

            def wslice(i):
                return wsb[:, i * BF:(i + 1) * BF]

            x2t = constp.tile([BF, FREE], bf16)
            h1t = constp.tile([BF, PW * PROWF], bf16)
            hot = constp.tile([BF, FREE], bf16)
            h1v = h1t[:].rearrange("p (r c) -> p r c", c=PROWF)

            # zero h1 padding borders once (persistent across reps/images)
            nc.gpsimd.memset(h1v[:, 0, :], 0.0)
            nc.gpsimd.memset(h1v[:, PW - 1, :], 0.0)
            nc.gpsimd.memset(h1v[:, 1:PW - 1, 0:NIMG], 0.0)
            nc.gpsimd.memset(h1v[:, 1:PW - 1, PROWF - NIMG:PROWF], 0.0)

            def conv1_block(b):
                lo, hi = 6 * b, min(6 * b + 6, H)
                for r in range(lo, hi):
                    ps = ps1p.tile([BF, ROWF], f32, name="ps1")
                    nc.tensor.matmul(
                        ps[:], wslice(0), x2t[:, r * ROWF:(r + 1) * ROWF],
                        start=True, stop=True)
                    nc.scalar.activation(
                        h1v[:, r + 1, NIMG:NIMG + ROWF], ps[:], PRELU,
                        bias=bsb[:, 0:1], scale=1.0, alpha=alpha1)

            nblocks = (H + 5) // 6

            for rep in range(reps):
                for k in range(NLD):
                    c0, c1 = k * FREE // NLD, (k + 1) * FREE // NLD
                    nc.sync.dma_start(x2t[:, c0:c1], x2[:, c0:c1])

                for b in range(min(3, nblocks)):
                    conv1_block(b)

                st_emitted = 0
                for g, (r0, n) in enumerate(groups):
                    pss = [ps2p.tile([BF, ROWF], f32, name="ps2") for i in range(n)]
                    for t, (dy, dx) in enumerate(TAP_ORDER):
                        for i in range(n):
                            r = r0 + i
                            off = (r + dy) * PROWF + dx * NIMG
                            nc.tensor.matmul(
                                pss[i][:], wslice(1 + t),
                                h1t[:, off:off + ROWF],
                                start=(t == 0), stop=(t == 8))
                    if g + 3 < nblocks:
                        conv1_block(g + 3)
                    for i in range(n):
                        r = r0 + i
                        nc.scalar.activation(
                            hot[:, r * ROWF:(r + 1) * ROWF], pss[i][:],
                            PRELU, bias=bsb[:, 1:2], scale=1.0, alpha=alpha2)
                    rows_done = r0 + n
                    while (st_emitted < NST
                           and rows_done >= (st_emitted + 1) * H // NST):
                        c0 = st_emitted * FREE // NST
                        c1 = (st_emitted + 1) * FREE // NST
                        getattr(nc, st_engine).dma_start(
                            out[:, c0:c1], hot[:, c0:c1])
                        st_emitted += 1
                assert st_emitted == NST

    if not nc.is_finalized():
        nc.finalize()
    return nc


def _prep_host(w2, bn1_g, bn1_b, bn1_m, bn1_v, wdw, bn2_g, bn2_b, bn2_m,
               bn2_v, w3, bn3_g, bn3_b, bn3_m, bn3_v):
    s1 = bn1_g / np.sqrt(bn1_v + EPS)
    t1 = bn1_b - bn1_m * s1
    s2 = bn2_g / np.sqrt(bn2_v + EPS)
    t2 = bn2_b - bn2_m * s2
    s3 = bn3_g / np.sqrt(bn3_v + EPS)
    t3 = bn3_b - bn3_m * s3
    w3p = w3 * s3[:, None]                  # [o,c] BN3-folded conv2 weights
    wdwp = wdw[:, 0] * s2[:, None, None]    # [c,3,3] BN2-folded dw weights

    wts = np.empty((BF, 10 * BF), np.float32)
    wts[:, 0:BF] = (w2 * s1[:, None]).T     # lhsT for conv1 (k=c_in, m=c_out)
    for ti, (dy, dx) in enumerate(TAP_ORDER):
        wts[:, (1 + ti) * BF:(2 + ti) * BF] = w3p.T * wdwp[:, dy, dx][:, None]

    bias = np.empty((BF, 2), np.float32)
    bias[:, 0] = t1
    bias[:, 1] = t3 + w3p @ t2
    return (np.ascontiguousarray(wts).astype(ml_dtypes.bfloat16),
            np.ascontiguousarray(bias))


def _run(inputs, trace=False, trace_kwargs=None, reps=1, variant="full"):
    from concourse.bass_utils import run_bass_kernel_spmd

    a1 = float(np.asarray(inputs["alpha1"]).reshape(-1)[0])
    a2 = float(np.asarray(inputs["alpha2"]).reshape(-1)[0])
    key = (a1, a2, reps, variant)
    if key not in _CACHE:
        _CACHE[key] = _build(a1, a2, reps, variant)
    nc = _CACHE[key]

    wts, bias = _prep_host(*[np.asarray(inputs[k], np.float32) for k in (
        "w2", "bn1_g", "bn1_b", "bn1_m", "bn1_v", "wdw", "bn2_g", "bn2_b",
        "bn2_m", "bn2_v", "w3", "bn3_g", "bn3_b", "bn3_m", "bn3_v")])

    x = np.asarray(inputs["x"], np.float32)
    B = x.shape[0]
    assert B == NIMG * NCORES
    xr = x.reshape(NCORES, NIMG, 2 * BF, HW)
    # [core, 116, 3136, img] image-interleaved bf16
    x2h = xr[:, :, BF:, :].transpose(0, 2, 3, 1)
    x2h = np.ascontiguousarray(x2h).astype(ml_dtypes.bfloat16)
    x2h = x2h.reshape(NCORES, BF, FREE)

    in_maps = [
        {"x2": x2h[c], "wts": wts, "bias": bias}
        for c in range(NCORES)
    ]
    kw = {}
    if trace:
        kw["trace"] = True
        kw["trace_cores"] = list(range(NCORES))
        kw.update(trace_kwargs or {})
    res = run_bass_kernel_spmd(nc, in_maps, core_ids=list(range(NCORES)), **kw)

    y = np.stack([np.asarray(res.results[c]["out"]) for c in range(NCORES)])
    y = y.astype(np.float32).reshape(NCORES, BF, HW, NIMG)
    full = np.empty((NCORES, NIMG, 2 * BF, HW), np.float32)
    full[:, :, 1::2, :] = y.transpose(0, 3, 1, 2)
    full[:, :, 0::2, :] = xr[:, :, :BF, :]
    return full.reshape(B, 2 * BF, H, W), res


def kernel(**inputs) -> np.ndarray:
    full, _ = _run(inputs, trace=False)
    return full
